# revision 10
# baseline (speedup 1.0000x reference)
"""Binarized CNN forward pass on 8 TRN2 NeuronCores (data-parallel, batch 256).

Self-contained: kernel(**inputs) takes the full unsharded inputs (as produced
by the reference's setup_inputs) and returns the full [256, 10] output.

Strategy
--------
- Pure data parallelism: 32 images per core; binarized (+-1) weights are exact
  in fp16 and replicated to all cores.
- Single-pass fp16 matmuls (activations rounded to fp16, weights exact),
  fp32 PSUM accumulation. Raw conv outputs are kept in fp32: pre-BN values
  have |mean| >> std per channel, so rounding them is amplified by the BN
  normalization (measured: bf16 raw -> 6e-2 rel; fp32 raw + fp16 acts ->
  ~1e-22 rel).
- Training-mode BN: per conv layer and output-channel chunk, each core
  computes per-channel (mean, E[x^2]) via bn_stats/bn_aggr; cores AllGather
  the 1KB stat vectors and combine locally (cheaper + earlier than a tail
  AllReduce: chunks finish at different times, so all but the last chunk's
  collective hides behind remaining matmuls).
- maxpool commutes with the monotone BN+relu transform (a > 0): raw outputs
  are pooled first (on DVE, during the collective window) and transformed
  after.
- Activations/raw are contiguous per channel-chunk: [128, h, w, b] with
  b=32 innermost; matmuls cover TWO output rows per PSUM tile (N<=512),
  halving instruction count vs row-at-a-time.
- Transforms are emitted first-rows-first and interleaved across chunks so
  the next layer's matmuls start ~2us after the last collective lands.
- Layer 1 (cin=1) uses an im2col with K=9 taps on partitions, packed 4x into
  PE row-groups (tile_position) over quarters of the output rows.
"""
import numpy as np
from concourse import bacc, tile, mybir
from concourse.ap import AP
from concourse.bass_utils import run_bass_kernel_spmd

F16 = mybir.dt.float16
F32 = mybir.dt.float32
AF = mybir.ActivationFunctionType
ALU = mybir.AluOpType

N_CORES = 8
B = 32          # per-core batch
EPS = 1e-5

# l, cin, cout, hin (after any pool of prev layer), hout, pool_after
CONV = [
    (1, 1, 128, 28, 26, False),
    (2, 128, 128, 26, 24, False),
    (3, 128, 256, 24, 22, False),
    (4, 256, 256, 22, 20, True),
    (5, 256, 512, 10, 8, False),
    (6, 512, 512, 8, 6, True),
]

# layer-1 output-row groups for 4x PE row-group packing
L1_GROUPS = [(0, 7), (7, 6), (13, 7), (20, 6)]


def _halves(wout):
    """Split a row of wout*B output columns into <=512-col matmul chunks."""
    n = wout * B
    if n <= 512:
        return [(0, wout)]
    assert wout % 2 == 0
    return [(0, wout // 2), (wout // 2, wout // 2)]


def _wchunks(wout):
    """Split wout into pieces wn with 2*wn*B <= 512 (row-pair matmuls)."""
    out = []
    w0 = 0
    while w0 < wout:
        wn = min(8, wout - w0)
        out.append((w0, wn))
        w0 += wn
    return out


def build(upto=9, dbg=True, reps=1):
    """Build the Bass module. upto: 1..6 = stop after conv layer `upto` and
    emit its transformed activations as debug outputs (dbg=False: tiny output
    only, for timing bisection); 9 = full net. reps: repeat the whole body
    sequentially (differential timing: t(reps=a)-t(reps=b) ~ (a-b) bodies)."""
    nc = bacc.Bacc("TRN2", target_bir_lowering=False, debug=False,
                   num_devices=N_CORES)

    # ---- parameters (per-core shards / replicated weights)
    xp = nc.declare_dram_parameter("xp", [28 * 28, B], F16, isOutput=False)
    w1p = nc.declare_dram_parameter("w1p", [128, 128], F16, isOutput=False)
    wp = {}
    gp, bp = {}, {}
    for (l, cin, cout, hin, hout, pool) in CONV:
        if l >= 2:
            ci_ch, co_ch = cin // 128, cout // 128
            wp[l] = nc.declare_dram_parameter(
                f"w{l}p", [ci_ch * 128, co_ch * 9 * 128], F16, isOutput=False)
        gp[l] = nc.declare_dram_parameter(f"g{l}p", [cout, 1], F32, isOutput=False)
        bp[l] = nc.declare_dram_parameter(f"b{l}p", [cout, 1], F32, isOutput=False)
    fw1p = fw2p = fw3p = None
    if upto >= 9:
        fw1p = nc.declare_dram_parameter("fw1p", [36 * 128, 1024], F16, isOutput=False)
        fw2p = nc.declare_dram_parameter("fw2p", [8 * 128, 1024], F16, isOutput=False)
        fw3p = nc.declare_dram_parameter("fw3p", [8 * 128, 10], F16, isOutput=False)
        identp = nc.declare_dram_parameter("identp", [32, 32], F32, isOutput=False)
        out_ext = nc.declare_dram_parameter("out", [10, B], F32, isOutput=True)

    dbg_t = None
    if upto < 9:
        (_, _, cout, _, hout, pool) = CONV[upto - 1]
        ho = hout // 2 if pool else hout
        co_ch = cout // 128
        if dbg:
            dbg_t = nc.declare_dram_parameter(
                "dbg", [co_ch * 128, ho * ho * B], F16, isOutput=True)
        else:
            dbg_t = nc.declare_dram_parameter(
                "dbg", [128, B], F16, isOutput=True)

    # ---- DRAM scratch for BN stat all-gather (per layer, per co chunk)
    cc_in, cc_out = {}, {}
    for (l, cin, cout, hin, hout, pool) in CONV:
        if l > upto:
            break
        for co in range(cout // 128):
            cc_in[(l, co)] = nc.dram_tensor(f"cc_in{l}_{co}", [128, 2], F32)
            cc_out[(l, co)] = nc.dram_tensor(f"cc_out{l}_{co}", [8 * 128, 2],
                                             F32, addr_space="Shared")

    with tile.TileContext(nc) as tc:
        const_pool = tc.alloc_tile_pool(name="const", bufs=1, side="left")
        stage_pool = tc.alloc_tile_pool(name="stage", bufs=1, side="right")
        psum_pool = tc.alloc_tile_pool(name="psum", bufs=8, space="PSUM")

        def body():
            eps_t = const_pool.tile([128, 1], F32, tag="eps")
            nc.vector.memset(eps_t[:], EPS)

            # gamma/beta for every layer, loaded up front: [128, 2*co_ch]
            gb_sb = {}
            for (l, cin, cout, hin, hout, pool) in CONV:
                if l > upto:
                    break
                co_ch = cout // 128
                gb_all = const_pool.tile([128, 2 * co_ch], F32, tag=f"gb{l}",
                                         name=f"gb{l}")
                nc.sync.dma_start(out=gb_all[:, :co_ch],
                                  in_=AP(gp[l], 0, [[1, 128], [128, co_ch]]))
                nc.sync.dma_start(out=gb_all[:, co_ch:],
                                  in_=AP(bp[l], 0, [[1, 128], [128, co_ch]]))
                gb_sb[l] = gb_all

            # persistent parity slots for next-layer co0 weight prefetch
            wf = {}

            def prefetch_w(nl):
                """Prefetch layer nl's (ci, co=0) weight tiles into parity
                slots (emitted during layer nl-1, so layer nl's first psum
                group never waits on HBM)."""
                if nl > min(upto, 6) or nl < 2:
                    return
                nci = CONV[nl - 1][1] // 128
                for ci in range(nci):
                    t = const_pool.tile([128, 9 * 128], F16,
                                        tag=f"wf{nl % 2}_{ci}",
                                        name=f"wf{nl}_{ci}")
                    nc.sync.dma_start(
                        out=t[:],
                        in_=wp[nl][ci * 128:(ci + 1) * 128, 0:1152])
                    wf[(nl, ci)] = t

            def conv_layer(l, act_in, act_pool_in):
                """act_in: dict ci -> contiguous tile [128, hin*win*B] fp16
                (layer 1: the packed im2col tile under key 0).
                Returns (act_out dict co -> tile, act_pool_out)."""
                (_, cin, cout, hin, hout, pool) = CONV[l - 1]
                ci_ch, co_ch = (cin // 128, cout // 128) if l >= 2 else (1, 1)
                win = hin
                wout = hout
                wch = _wchunks(wout)
                if l == 1:
                    halves = _halves(wout)
                    ntiles = wout * len(halves)
                else:
                    ntiles = (hout // 2) * len(wch)

                # right stack: stats+raw first (live through transform),
                # weights LAST (released right after the matmuls)
                stats_pool = tc.alloc_tile_pool(name=f"st{l}", bufs=1,
                                                side="right")
                raw_pool = tc.alloc_tile_pool(name=f"raw{l}", bufs=1,
                                              side="right")
                wpool = tc.alloc_tile_pool(name=f"w{l}", bufs=1, side="right")

                w_sb = {}
                if l >= 2:
                    for co in range(co_ch):
                        for ci in range(ci_ch):
                            if co == 0 and (l, ci) in wf:
                                w_sb[(ci, 0)] = wf[(l, ci)]
                                continue
                            t = wpool.tile([128, 9 * 128], F16,
                                           tag=f"w{ci}_{co}",
                                           name=f"w{l}_{ci}_{co}")
                            nc.sync.dma_start(
                                out=t[:],
                                in_=wp[l][ci * 128:(ci + 1) * 128,
                                          co * 1152:(co + 1) * 1152])
                            w_sb[(ci, co)] = t
                else:
                    t = wpool.tile([128, 128], F16, tag="w1", name="w1sb")
                    nc.sync.dma_start(out=t[:], in_=w1p[:, :])
                    w_sb[(0, 0)] = t
                prefetch_w(l + 1)
                gb_all = gb_sb[l]

                stats = {}
                raw = {}
                for co in range(co_ch):
                    stats[co] = stats_pool.tile([128, ntiles * 6], F32,
                                                tag=f"s{co}",
                                                name=f"stats{l}_{co}")
                    raw[co] = raw_pool.tile([128, hout * wout * B], F32,
                                            tag=f"r{co}", name=f"raw{l}_{co}")

                cc_sb = const_pool.tile([128, 2 * co_ch], F32, tag=f"cc{l}",
                                        name=f"cc{l}")

                def stats_and_gather(co):
                    # per-core (mean, var) -> (mean, var+mean^2) -> AllGather
                    nc.vector.bn_aggr(out=cc_sb[:, co * 2:co * 2 + 2],
                                      in_=stats[co][:])
                    nc.vector.scalar_tensor_tensor(
                        out=cc_sb[:, co * 2 + 1:co * 2 + 2],
                        in0=cc_sb[:, co * 2:co * 2 + 1],
                        scalar=cc_sb[:, co * 2:co * 2 + 1],
                        in1=cc_sb[:, co * 2 + 1:co * 2 + 2],
                        op0=ALU.mult, op1=ALU.add)
                    nc.sync.dma_start(out=cc_in[(l, co)][:, :],
                                      in_=cc_sb[:, co * 2:co * 2 + 2])
                    nc.gpsimd.collective_compute(
                        "AllGather", ALU.bypass,
                        replica_groups=[list(range(N_CORES))],
                        ins=[cc_in[(l, co)][:, :]],
                        outs=[cc_out[(l, co)][:, :]])

                # ---- matmuls (co-outer so each chunk's collective hides
                # behind the next chunk's matmuls)
                if l == 1:
                    ic = act_in[0]
                    for gi, (r0, nr) in enumerate(L1_GROUPS):
                        for rr in range(nr):
                            for hidx, (w0, wn) in enumerate(halves):
                                n = wn * B
                                ps = psum_pool.tile([128, n], F32, tag="ps",
                                                    name="ps1")
                                off = (rr * 26 + w0) * B
                                lhsT = w_sb[(0, 0)][32 * gi:32 * gi + 9, :]
                                nc.tensor.matmul(
                                    out=ps[:], lhsT=lhsT,
                                    rhs=ic[32 * gi:32 * gi + 9, off:off + n],
                                    start=True, stop=True,
                                    tile_position=(32 * gi, 0))
                                r = r0 + rr
                                tidx = r * len(halves) + hidx
                                nc.vector.bn_stats(
                                    out=stats[0][:, tidx * 6:(tidx + 1) * 6],
                                    in_=ps[:])
                                nc.scalar.copy(
                                    out=raw[0][:, (r * 26 + w0) * B:
                                               (r * 26 + w0) * B + n],
                                    in_=ps[:])
                    stats_and_gather(0)
                else:
                    views = {ci: act_in[ci][:].rearrange(
                        "p (h w b) -> p h w b", h=hin, w=win, b=B)
                        for ci in range(ci_ch)}
                    rviews = {co: raw[co][:].rearrange(
                        "p (h w b) -> p h w b", h=hout, w=wout, b=B)
                        for co in range(co_ch)}
                    for co in range(co_ch):
                        for rp in range(hout // 2):
                            r = 2 * rp
                            for hidx, (w0, wn) in enumerate(wch):
                                n = 2 * wn * B
                                ps = psum_pool.tile([128, n], F32, tag="ps",
                                                    name="psc")
                                nmm = ci_ch * 9
                                k = 0
                                for ci in range(ci_ch):
                                    v = views[ci]
                                    for dy in range(3):
                                        for dx in range(3):
                                            rhs = v[:, r + dy:r + dy + 2,
                                                    w0 + dx:w0 + dx + wn, :]
                                            t = dy * 3 + dx
                                            lhsT = w_sb[(ci, co)][
                                                :, t * 128:(t + 1) * 128]
                                            nc.tensor.matmul(
                                                out=ps[:], lhsT=lhsT, rhs=rhs,
                                                start=(k == 0),
                                                stop=(k == nmm - 1))
                                            k += 1
                                tidx = rp * len(wch) + hidx
                                nc.vector.bn_stats(
                                    out=stats[co][:, tidx * 6:(tidx + 1) * 6],
                                    in_=ps[:])
                                nc.scalar.copy(
                                    out=rviews[co][:, r:r + 2,
                                                   w0:w0 + wn, :],
                                    in_=ps[:])
                        stats_and_gather(co)

                wpool.release()

                # ---- input activations are dead; swap left-stack pools
                if act_pool_in is not None:
                    act_pool_in.release()
                act_pool_out = tc.alloc_tile_pool(name=f"act{l}", bufs=1,
                                                  side="left")

                # ---- pools on DVE: depend only on raw, so they run during
                # the last chunk's collective. pooled/pm tiles reuse the
                # region wpool just freed.
                pooled = {}
                pm_pool = None
                if pool:
                    pm_pool = tc.alloc_tile_pool(name=f"pm{l}", bufs=1,
                                                 side="right")
                    for co in range(co_ch):
                        pooled[co] = pm_pool.tile(
                            [128, (hout // 2) * (wout // 2) * B], F32,
                            tag=f"p{co}", name=f"pool{l}_{co}")
                    pm = pm_pool.tile([128, (hout // 2) * wout * B],
                                      F32, tag="pm", name=f"pm{l}")
                    for co in range(co_ch):
                        rv = raw[co][:].rearrange(
                            "p (h two w b) -> p h two w b",
                            h=hout // 2, two=2, w=wout, b=B)
                        nc.vector.tensor_max(out=pm[:], in0=rv[:, :, 0, :, :],
                                             in1=rv[:, :, 1, :, :])
                        pv = pm[:].rearrange(
                            "p (h w two b) -> p h w two b",
                            h=hout // 2, w=wout // 2, two=2, b=B)
                        nc.vector.tensor_max(out=pooled[co][:],
                                             in0=pv[:, :, :, 0, :],
                                             in1=pv[:, :, :, 1, :])

                # ---- per chunk: gathered stats -> local combine -> a, c ->
                # transforms. Interleaved per chunk so chunk co's transforms
                # are never queued behind a later chunk's collective wait:
                # chunks 0..co_ch-2 drain during the last chunk's collective.
                a_all = const_pool.tile([128, co_ch], F32, tag=f"a{l}",
                                        name=f"a{l}")
                c_all = const_pool.tile([128, co_ch], F32, tag=f"c{l}",
                                        name=f"c{l}")
                g_t = const_pool.tile([128, 16 * co_ch], F32, tag=f"gt{l}",
                                      name=f"g{l}")
                u_t = const_pool.tile([128, 8 * co_ch], F32, tag=f"ut{l}",
                                      name=f"u{l}")
                ho = hout // 2 if pool else hout
                wo = wout // 2 if pool else wout
                src = pooled if pool else raw
                act_out = {}
                for co in range(co_ch):
                    act_out[co] = act_pool_out.tile([128, ho * wo * B], F16,
                                                    tag=f"h{co}",
                                                    name=f"a{l}_{co}")
                # row blocks: (start, nrows)
                blocks = []
                r = 0
                while r < ho:
                    nr = min(4, ho - r)
                    blocks.append((r, nr))
                    r += nr
                for co in range(co_ch):
                    g_c = g_t[:, co * 16:(co + 1) * 16]
                    u_c = u_t[:, co * 8:(co + 1) * 8]
                    nc.sync.dma_start(
                        out=g_c,
                        in_=AP(cc_out[(l, co)], 0,
                               [[2, 128], [256, 8], [1, 2]]))
                    # sum the 8 gathered (mean, E2) pairs: 16 -> 8 -> 4 -> 2
                    nc.vector.tensor_add(out=u_c, in0=g_c[:, 0:8],
                                         in1=g_c[:, 8:16])
                    nc.vector.tensor_add(out=u_c[:, 0:4], in0=u_c[:, 0:4],
                                         in1=u_c[:, 4:8])
                    nc.vector.tensor_add(out=u_c[:, 0:2], in0=u_c[:, 0:2],
                                         in1=u_c[:, 2:4])
                    s1 = u_c[:, 0:1]
                    s2 = u_c[:, 1:2]
                    # nm = -mean; nvar = mean^2 - E2 = -var
                    nm = u_c[:, 2:3]
                    nvar = u_c[:, 3:4]
                    nc.vector.tensor_scalar_mul(nm, s1, -1.0 / N_CORES)
                    nc.vector.tensor_scalar_mul(nvar, s2, 1.0 / N_CORES)
                    # nvar := (nm*nm) - E2/8 = mean^2 - E[x^2] = -var
                    nc.vector.scalar_tensor_tensor(
                        out=nvar, in0=nm, scalar=nm, in1=nvar,
                        op0=ALU.mult, op1=ALU.subtract)
                    sd = u_c[:, 4:5]
                    nc.scalar.activation(out=sd, in_=nvar, func=AF.Sqrt,
                                         bias=eps_t[:], scale=-1.0)
                    rc = u_c[:, 5:6]
                    nc.vector.reciprocal(out=rc, in_=sd)
                    a_c = a_all[:, co:co + 1]
                    c_c = c_all[:, co:co + 1]
                    nc.vector.tensor_mul(out=a_c, in0=rc,
                                         in1=gb_all[:, co:co + 1])
                    nc.vector.tensor_mul(out=c_c, in0=nm, in1=a_c)
                    nc.vector.tensor_add(out=c_c, in0=c_c,
                                         in1=gb_all[:, co_ch + co:
                                                    co_ch + co + 1])
                    for bi, (r0, nr) in enumerate(blocks):
                        s = src[co][:, r0 * wo * B:(r0 + nr) * wo * B]
                        d = act_out[co][:, r0 * wo * B:(r0 + nr) * wo * B]
                        if (bi + co) % 2 == 0:
                            nc.scalar.activation(out=d, in_=s, func=AF.Relu,
                                                 bias=c_c, scale=a_c)
                        else:
                            nc.vector.tensor_scalar(
                                out=d, in0=s, scalar1=a_c, scalar2=c_c,
                                op0=ALU.mult, op1=ALU.add)
                            nc.vector.tensor_scalar_max(d, d, 0.0)

                if pm_pool is not None:
                    pm_pool.release()
                raw_pool.release()
                stats_pool.release()
                return act_out, act_pool_out

            # ---- layer 1 im2col source (4 row-groups packed on partitions)
            im2col_pool = tc.alloc_tile_pool(name="im2col", bufs=1,
                                             side="left")
            ic = im2col_pool.tile([128, 7 * 26 * B], F16, tag="ic")
            for gi, (r0, nr) in enumerate(L1_GROUPS):
                for dy in range(3):
                    in_ap = AP(xp, ((r0 + dy) * 28) * B,
                               [[B, 3], [28 * B, nr], [B, 26], [1, B]])
                    nc.sync.dma_start(
                        out=ic[32 * gi + 3 * dy:32 * gi + 3 * dy + 3,
                               0:nr * 26 * B],
                        in_=in_ap)

            act = {0: ic}
            act, act_pool = conv_layer(1, act, im2col_pool)

            fc_a = None
            fw1_sb = []
            for l in range(2, min(upto, 6) + 1):
                if l == 5 and upto >= 9:
                    # preload fw1 behind L5/L6 compute
                    fc_a = tc.alloc_tile_pool(name="fc_a", bufs=1,
                                              side="right")
                    for kc in range(18):
                        t = fc_a.tile([128, 1024], F16, tag=f"fw1_{kc}",
                                      name=f"fw1_{kc}")
                        nc.sync.dma_start(
                            out=t[:], in_=fw1p[kc * 128:(kc + 1) * 128, :])
                        fw1_sb.append(t)
                act, act_pool = conv_layer(l, act, act_pool)

            if upto < 9:
                (_, _, cout, _, hout, pool) = CONV[upto - 1]
                ho = hout // 2 if pool else hout
                co_ch = cout // 128
                if not dbg:
                    nc.sync.dma_start(out=dbg_t[:, :], in_=act[0][:, :B])
                else:
                    for co in range(co_ch):
                        nc.sync.dma_start(
                            out=dbg_t[co * 128:(co + 1) * 128, :],
                            in_=act[co][:])
            else:
                # ---------------- FC stack ----------------
                fc_pool = tc.alloc_tile_pool(name="fc", bufs=1, side="right")
                for kc in range(18, 36):
                    t = fc_pool.tile([128, 1024], F16, tag=f"fw1_{kc}",
                                     name=f"fw1_{kc}")
                    nc.sync.dma_start(out=t[:],
                                      in_=fw1p[kc * 128:(kc + 1) * 128, :])
                    fw1_sb.append(t)
                fw2_sb = []
                for kc in range(8):
                    t = fc_pool.tile([128, 1024], F16, tag=f"fw2_{kc}",
                                     name=f"fw2_{kc}")
                    nc.sync.dma_start(out=t[:],
                                      in_=fw2p[kc * 128:(kc + 1) * 128, :])
                    fw2_sb.append(t)
                fw3_sb = []
                for kc in range(8):
                    t = fc_pool.tile([128, 10], F16, tag=f"fw3_{kc}",
                                     name=f"fw3_{kc}")
                    nc.sync.dma_start(out=t[:],
                                      in_=fw3p[kc * 128:(kc + 1) * 128, :])
                    fw3_sb.append(t)

                ident_sb = fc_pool.tile([32, 32], F32, tag="ident",
                                        name="ident")
                nc.sync.dma_start(out=ident_sb[:], in_=identp[:, :])

                # fc1 (transposed form): out[b, o] with the activation chunk
                # as the stationary operand and fw1 rows streaming (N=512) --
                # 72 big matmuls instead of 288 N=32 ones.
                # lhsT kc = (co*3 + r)*3 + w: act6 co tiles [128, 9*B]
                lhs1 = []
                for co in range(4):
                    for rw in range(9):
                        lhs1.append(act[co][:, rw * B:(rw + 1) * B])

                def fc_T(lhs_list, w_sb, n_out, tagp):
                    """Transposed-form FC + relu + PE transpose back.
                    lhs_list: kc -> [128, 32] fp16 chunks of the input (k on
                    partitions). w_sb: kc -> [128, n_out] weight rows (rhs).
                    Returns list over 8 o-chunks of [128, 32] fp16 tiles
                    (o on partitions)."""
                    nkc = len(lhs_list)
                    ysb = fc_pool.tile([32, n_out], F32, tag=f"{tagp}y",
                                       name=f"{tagp}y")
                    for oh in range(n_out // 512):
                        ps = psum_pool.tile([32, 512], F32, tag="ps",
                                            name=f"{tagp}ps{oh}")
                        for kc, lh in enumerate(lhs_list):
                            nc.tensor.matmul(
                                out=ps[:], lhsT=lh,
                                rhs=w_sb[kc][:, oh * 512:(oh + 1) * 512],
                                start=(kc == 0), stop=(kc == nkc - 1))
                        nc.scalar.activation(
                            out=ysb[:, oh * 512:(oh + 1) * 512], in_=ps[:],
                            func=AF.Relu)
                    outs = []
                    for kc2 in range(n_out // 128):
                        trp = psum_pool.tile([128, 32], F32, tag="ps",
                                             name=f"{tagp}tr{kc2}")
                        nc.tensor.transpose(
                            out=trp[:],
                            in_=ysb[:, kc2 * 128:(kc2 + 1) * 128],
                            identity=ident_sb[:])
                        t = fc_pool.tile([128, 32], F16, tag=f"{tagp}T{kc2}",
                                         name=f"{tagp}T{kc2}")
                        nc.vector.tensor_copy(out=t[:], in_=trp[:])
                        outs.append(t)
                    return outs

                y1T = fc_T(lhs1, fw1_sb, 1024, "y1")
                y2T = fc_T([t[:] for t in y1T], fw2_sb, 1024, "y2")

                ps = psum_pool.tile([10, B], F32, tag="ps", name="ps10")
                for kc, rh in enumerate([t[:] for t in y2T]):
                    lhsT = fw3_sb[kc][:, :]
                    nc.tensor.matmul(out=ps[:], lhsT=lhsT, rhs=rh,
                                     start=(kc == 0), stop=(kc == 7))
                sig = fc_pool.tile([10, B], F32, tag="sig", name="sig")
                nc.scalar.activation(out=sig[:], in_=ps[:], func=AF.Sigmoid)
                nc.sync.dma_start(out=out_ext[:, :], in_=sig[:])
                fc_pool.release()
                if fc_a is not None:
                    fc_a.release()
            act_pool.release()

        for _rep in range(reps):
            body()

        psum_pool.release()
        stage_pool.release()
        const_pool.release()

    nc.compile()
    return nc


# ---------------- host-side input prep ----------------

def _f16(x):
    return np.asarray(x, np.float32).astype(np.float16)


def prep_inputs(inputs):
    """Full reference inputs -> per-core input maps."""
    x = np.asarray(inputs["x"], np.float32).reshape(256, 28 * 28)
    shared = {}
    w1 = np.sign(np.asarray(inputs["w1"], np.float32)).reshape(128, 9)
    w1rep = np.zeros((128, 128), np.float32)
    for gi in range(4):
        w1rep[32 * gi:32 * gi + 9, :] = w1.T
    shared["w1p"] = _f16(w1rep)
    for l in (2, 3, 4, 5, 6):
        w = np.sign(np.asarray(inputs[f"w{l}"], np.float32))
        cout, cin = w.shape[0], w.shape[1]
        ci_ch, co_ch = cin // 128, cout // 128
        s = w.reshape(co_ch, 128, ci_ch, 128, 9)
        s = np.transpose(s, (2, 3, 0, 4, 1))  # [ci_ch,128,co_ch,9,128]
        shared[f"w{l}p"] = _f16(s.reshape(ci_ch * 128, co_ch * 9 * 128).copy())
    for l in range(1, 7):
        shared[f"g{l}p"] = np.asarray(inputs[f"g{l}"], np.float32).reshape(-1, 1)
        shared[f"b{l}p"] = np.asarray(inputs[f"b{l}"], np.float32).reshape(-1, 1)
    fw1 = np.sign(np.asarray(inputs["fw1"], np.float32))  # [1024, 4608]
    v = fw1.reshape(1024, 4, 128, 9)
    v = np.transpose(v, (1, 3, 2, 0))  # [4, 9, 128, 1024], kc = cc*9+hw
    shared["fw1p"] = _f16(v.reshape(36 * 128, 1024).copy())
    fw2 = np.sign(np.asarray(inputs["fw2"], np.float32))
    shared["fw2p"] = _f16(fw2.T.reshape(8 * 128, 1024).copy())
    fw3 = np.sign(np.asarray(inputs["fw3"], np.float32))
    shared["fw3p"] = _f16(fw3.T.reshape(8 * 128, 10).copy())
    shared["identp"] = np.eye(32, dtype=np.float32)

    per_core = []
    for c in range(N_CORES):
        xs = x[c * B:(c + 1) * B].T.copy()  # [784, B]
        m = dict(shared)
        m["xp"] = _f16(xs)
        per_core.append(m)
    return per_core


_NC_CACHE = {}


def kernel(**inputs) -> np.ndarray:
    if "nc" not in _NC_CACHE:
        _NC_CACHE["nc"] = build(upto=9)
    nc = _NC_CACHE["nc"]
    per_core = prep_inputs(inputs)
    last_err = None
    for _attempt in range(3):
        try:
            res = run_bass_kernel_spmd(nc, per_core, list(range(N_CORES)))
            break
        except Exception as e:  # transient device wedge: wait and retry
            last_err = e
            import time as _time
            _time.sleep(15)
    else:
        raise last_err
    out = np.empty((256, 10), np.float32)
    for c in range(N_CORES):
        out[c * B:(c + 1) * B] = res.results[c]["out"].T
    return out


if __name__ == "__main__":
    import reference as R
    inputs = R.setup_inputs()
    got = kernel(**{k: np.asarray(v) for k, v in inputs.items()})
    exp = np.asarray(R.reference(**inputs))
    err = np.abs(got - exp)
    rel = np.linalg.norm(got - exp) / np.linalg.norm(exp)
    print(f"absmax {err.max():.3e}  rel {rel:.3e}")


# revision 16
# speedup vs baseline: 3.2320x; 3.2320x over previous
"""Binarized CNN forward pass on 8 TRN2 NeuronCores (data-parallel, batch 256).

Self-contained: kernel(**inputs) takes the full unsharded inputs (as produced
by the reference's setup_inputs) and returns the full [256, 10] output.

Strategy
--------
- Pure data parallelism: 32 images per core; binarized (+-1) weights are exact
  in fp16 and replicated to all cores.
- Single-pass fp16 matmuls (activations rounded to fp16, weights exact),
  fp32 PSUM accumulation. Raw conv outputs are kept in fp32: pre-BN values
  have |mean| >> std per channel, so rounding them is amplified by the BN
  normalization (measured: bf16 raw -> 6e-2 rel; fp32 raw + fp16 acts ->
  ~1e-22 rel).
- Training-mode BN: per conv layer and output-channel chunk, each core
  computes per-channel (mean, E[x^2]) via bn_stats/bn_aggr; cores AllGather
  the 1KB stat vectors and combine locally (cheaper + earlier than a tail
  AllReduce: chunks finish at different times, so all but the last chunk's
  collective hides behind remaining matmuls).
- maxpool commutes with the monotone BN+relu transform (a > 0): raw outputs
  are pooled first (on DVE, during the collective window) and transformed
  after.
- Activations/raw are contiguous per channel-chunk: [128, h, w, b] with
  b=32 innermost; matmuls cover TWO output rows per PSUM tile (N<=512),
  halving instruction count vs row-at-a-time.
- Transforms are emitted first-rows-first and interleaved across chunks so
  the next layer's matmuls start ~2us after the last collective lands.
- Layer 1 (cin=1) uses an im2col with K=9 taps on partitions, packed 4x into
  PE row-groups (tile_position) over quarters of the output rows.
"""
import numpy as np
from concourse import bacc, tile, mybir
from concourse.ap import AP
from concourse.bass_utils import run_bass_kernel_spmd

F16 = mybir.dt.float16
F32 = mybir.dt.float32
AF = mybir.ActivationFunctionType
ALU = mybir.AluOpType

N_CORES = 8
B = 32          # per-core batch
EPS = 1e-5

# l, cin, cout, hin (after any pool of prev layer), hout, pool_after
CONV = [
    (1, 1, 128, 28, 26, False),
    (2, 128, 128, 26, 24, False),
    (3, 128, 256, 24, 22, False),
    (4, 256, 256, 22, 20, True),
    (5, 256, 512, 10, 8, False),
    (6, 512, 512, 8, 6, True),
]

# layer-1 output-row groups for 4x PE row-group packing
L1_GROUPS = [(0, 7), (7, 6), (13, 7), (20, 6)]


def _halves(wout):
    """Split a row of wout*B output columns into <=512-col matmul chunks."""
    n = wout * B
    if n <= 512:
        return [(0, wout)]
    assert wout % 2 == 0
    return [(0, wout // 2), (wout // 2, wout // 2)]


def _wchunks(wout):
    """Split wout into pieces wn with 2*wn*B <= 512 (row-pair matmuls)."""
    out = []
    w0 = 0
    while w0 < wout:
        wn = min(8, wout - w0)
        out.append((w0, wn))
        w0 += wn
    return out


def build(upto=9, dbg=True, reps=1, collectives=True):
    """Build the Bass module. upto: 1..6 = stop after conv layer `upto` and
    emit its transformed activations as debug outputs (dbg=False: tiny output
    only, for timing bisection); 9 = full net. reps: repeat the whole body
    sequentially (differential timing: t(reps=a)-t(reps=b) ~ (a-b) bodies)."""
    nc = bacc.Bacc("TRN2", target_bir_lowering=False, debug=False,
                   num_devices=N_CORES)

    # ---- parameters (per-core shards / replicated weights)
    xp = nc.declare_dram_parameter("xp", [28 * 28, B], F16, isOutput=False)
    w1p = nc.declare_dram_parameter("w1p", [128, 128], F16, isOutput=False)
    wp = {}
    gp, bp = {}, {}
    for (l, cin, cout, hin, hout, pool) in CONV:
        if l >= 2:
            ci_ch, co_ch = cin // 128, cout // 128
            wp[l] = nc.declare_dram_parameter(
                f"w{l}p", [ci_ch * 128, co_ch * 9 * 128], F16, isOutput=False)
        gp[l] = nc.declare_dram_parameter(f"g{l}p", [cout, 1], F32, isOutput=False)
        bp[l] = nc.declare_dram_parameter(f"b{l}p", [cout, 1], F32, isOutput=False)
    fw1p = fw2p = fw3p = None
    if upto >= 9:
        fw1p = nc.declare_dram_parameter("fw1p", [36 * 128, 1024], F16, isOutput=False)
        fw2p = nc.declare_dram_parameter("fw2p", [8 * 128, 1024], F16, isOutput=False)
        fw3p = nc.declare_dram_parameter("fw3p", [8 * 128, 10], F16, isOutput=False)
        identp = nc.declare_dram_parameter("identp", [32, 32], F32, isOutput=False)
        out_ext = nc.declare_dram_parameter("out", [10, B], F32, isOutput=True)

    dbg_t = None
    if upto < 9:
        (_, _, cout, _, hout, pool) = CONV[upto - 1]
        ho = hout // 2 if pool else hout
        co_ch = cout // 128
        if dbg:
            dbg_t = nc.declare_dram_parameter(
                "dbg", [co_ch * 128, ho * ho * B], F16, isOutput=True)
        else:
            dbg_t = nc.declare_dram_parameter(
                "dbg", [128, B], F16, isOutput=True)

    # ---- DRAM scratch for BN stat all-gather (per layer, per co chunk)
    cc_in, cc_out = {}, {}
    for (l, cin, cout, hin, hout, pool) in CONV:
        if l > upto:
            break
        for co in range(cout // 128):
            cc_in[(l, co)] = nc.dram_tensor(f"cc_in{l}_{co}", [128, 2], F32)
            cc_out[(l, co)] = nc.dram_tensor(f"cc_out{l}_{co}", [8 * 128, 2],
                                             F32, addr_space="Shared")

    with tile.TileContext(nc) as tc:
        const_pool = tc.alloc_tile_pool(name="const", bufs=1, side="left")
        stage_pool = tc.alloc_tile_pool(name="stage", bufs=1, side="right")
        psum_pool = tc.alloc_tile_pool(name="psum", bufs=8, space="PSUM")

        def body():
            eps_t = const_pool.tile([128, 1], F32, tag="eps")
            nc.vector.memset(eps_t[:], EPS)

            w1f = const_pool.tile([128, 128], F16, tag="w1f", name="w1f")
            nc.sync.dma_start(out=w1f[:], in_=w1p[:, :])

            # ---- layer 1 im2col source (4 row-groups packed on partitions)
            im2col_pool = tc.alloc_tile_pool(name="im2col", bufs=1,
                                             side="left")
            ic = im2col_pool.tile([128, 7 * 26 * B], F16, tag="ic")
            for gi, (r0, nr) in enumerate(L1_GROUPS):
                for dy in range(3):
                    in_ap = AP(xp, ((r0 + dy) * 28) * B,
                               [[B, 3], [28 * B, nr], [B, 26], [1, B]])
                    nc.sync.dma_start(
                        out=ic[32 * gi + 3 * dy:32 * gi + 3 * dy + 3,
                               0:nr * 26 * B],
                        in_=in_ap)

            # gamma/beta for every layer, loaded up front: [128, 2*co_ch]
            gb_sb = {}
            for (l, cin, cout, hin, hout, pool) in CONV:
                if l > upto:
                    break
                co_ch = cout // 128
                gb_all = const_pool.tile([128, 2 * co_ch], F32, tag=f"gb{l}",
                                         name=f"gb{l}")
                nc.sync.dma_start(out=gb_all[:, :co_ch],
                                  in_=AP(gp[l], 0, [[1, 128], [128, co_ch]]))
                nc.sync.dma_start(out=gb_all[:, co_ch:],
                                  in_=AP(bp[l], 0, [[1, 128], [128, co_ch]]))
                gb_sb[l] = gb_all

            # persistent parity slots for next-layer co0 weight prefetch
            wf = {}

            def prefetch_w(nl):
                """Prefetch layer nl's (ci, co=0) weight tiles into parity
                slots (emitted during layer nl-1, so layer nl's first psum
                group never waits on HBM)."""
                if nl > min(upto, 6) or nl < 2:
                    return
                nci = CONV[nl - 1][1] // 128
                for ci in range(nci):
                    t = const_pool.tile([128, 9 * 128], F16,
                                        tag=f"wf{nl % 2}_{ci}",
                                        name=f"wf{nl}_{ci}")
                    nc.sync.dma_start(
                        out=t[:],
                        in_=wp[nl][ci * 128:(ci + 1) * 128, 0:1152])
                    wf[(nl, ci)] = t

            def conv_layer(l, act_in, act_pool_in, post_w_hook=None):
                """act_in: dict ci -> contiguous tile [128, hin*win*B] fp16
                (layer 1: the packed im2col tile under key 0).
                Returns (act_out dict co -> tile, act_pool_out)."""
                (_, cin, cout, hin, hout, pool) = CONV[l - 1]
                ci_ch, co_ch = (cin // 128, cout // 128) if l >= 2 else (1, 1)
                win = hin
                wout = hout
                wch = _wchunks(wout)
                if l == 1:
                    halves = _halves(wout)
                    ntiles = wout * len(halves)
                else:
                    ntiles = (hout // 2) * len(wch)

                # right stack: stats+raw first (live through transform),
                # weights LAST (released right after the matmuls)
                stats_pool = tc.alloc_tile_pool(name=f"st{l}", bufs=1,
                                                side="right")
                raw_pool = tc.alloc_tile_pool(name=f"raw{l}", bufs=1,
                                              side="right")
                wpool = tc.alloc_tile_pool(name=f"w{l}", bufs=1, side="right")

                w_sb = {}
                if l >= 2:
                    for co in range(co_ch):
                        for ci in range(ci_ch):
                            if co == 0 and (l, ci) in wf:
                                w_sb[(ci, 0)] = wf[(l, ci)]
                                continue
                            t = wpool.tile([128, 9 * 128], F16,
                                           tag=f"w{ci}_{co}",
                                           name=f"w{l}_{ci}_{co}")
                            nc.sync.dma_start(
                                out=t[:],
                                in_=wp[l][ci * 128:(ci + 1) * 128,
                                          co * 1152:(co + 1) * 1152])
                            w_sb[(ci, co)] = t
                else:
                    w_sb[(0, 0)] = w1f
                prefetch_w(l + 1)
                if post_w_hook is not None:
                    post_w_hook()
                gb_all = gb_sb[l]

                stats = {}
                raw = {}
                for co in range(co_ch):
                    stats[co] = stats_pool.tile([128, ntiles * 6], F32,
                                                tag=f"s{co}",
                                                name=f"stats{l}_{co}")
                    raw[co] = raw_pool.tile([128, hout * wout * B], F32,
                                            tag=f"r{co}", name=f"raw{l}_{co}")

                cc_sb = const_pool.tile([128, 2 * co_ch], F32, tag=f"cc{l}",
                                        name=f"cc{l}")

                def stats_and_gather(co):
                    # per-core (mean, var) -> AllGather (the var+mean^2
                    # correction happens post-gather, off the pre-AG path)
                    nc.vector.bn_aggr(out=cc_sb[:, co * 2:co * 2 + 2],
                                      in_=stats[co][:])
                    nc.sync.dma_start(out=cc_in[(l, co)][:, :],
                                      in_=cc_sb[:, co * 2:co * 2 + 2])
                    if collectives:
                        nc.gpsimd.collective_compute(
                            "AllGather", ALU.bypass,
                            replica_groups=[list(range(N_CORES))],
                            ins=[cc_in[(l, co)][:, :]],
                            outs=[cc_out[(l, co)][:, :]])

                # ---- matmuls (co-outer so each chunk's collective hides
                # behind the next chunk's matmuls)
                if l == 1:
                    ic = act_in[0]
                    for gi, (r0, nr) in enumerate(L1_GROUPS):
                        for rr in range(nr):
                            for hidx, (w0, wn) in enumerate(halves):
                                n = wn * B
                                ps = psum_pool.tile([128, n], F32, tag="ps",
                                                    name="ps1")
                                off = (rr * 26 + w0) * B
                                lhsT = w_sb[(0, 0)][32 * gi:32 * gi + 9, :]
                                nc.tensor.matmul(
                                    out=ps[:], lhsT=lhsT,
                                    rhs=ic[32 * gi:32 * gi + 9, off:off + n],
                                    start=True, stop=True,
                                    tile_position=(32 * gi, 0))
                                r = r0 + rr
                                tidx = r * len(halves) + hidx
                                nc.vector.bn_stats(
                                    out=stats[0][:, tidx * 6:(tidx + 1) * 6],
                                    in_=ps[:])
                                nc.scalar.copy(
                                    out=raw[0][:, (r * 26 + w0) * B:
                                               (r * 26 + w0) * B + n],
                                    in_=ps[:])
                    stats_and_gather(0)
                else:
                    views = {ci: act_in[ci][:].rearrange(
                        "p (h w b) -> p h w b", h=hin, w=win, b=B)
                        for ci in range(ci_ch)}
                    rviews = {co: raw[co][:].rearrange(
                        "p (h w b) -> p h w b", h=hout, w=wout, b=B)
                        for co in range(co_ch)}
                    for co in range(co_ch):
                        for rp in range(hout // 2):
                            r = 2 * rp
                            for hidx, (w0, wn) in enumerate(wch):
                                n = 2 * wn * B
                                ps = psum_pool.tile([128, n], F32, tag="ps",
                                                    name="psc")
                                nmm = ci_ch * 9
                                k = 0
                                for ci in range(ci_ch):
                                    v = views[ci]
                                    for dy in range(3):
                                        for dx in range(3):
                                            rhs = v[:, r + dy:r + dy + 2,
                                                    w0 + dx:w0 + dx + wn, :]
                                            t = dy * 3 + dx
                                            lhsT = w_sb[(ci, co)][
                                                :, t * 128:(t + 1) * 128]
                                            nc.tensor.matmul(
                                                out=ps[:], lhsT=lhsT, rhs=rhs,
                                                start=(k == 0),
                                                stop=(k == nmm - 1))
                                            k += 1
                                tidx = rp * len(wch) + hidx
                                nc.vector.bn_stats(
                                    out=stats[co][:, tidx * 6:(tidx + 1) * 6],
                                    in_=ps[:])
                                nc.scalar.copy(
                                    out=rviews[co][:, r:r + 2,
                                                   w0:w0 + wn, :],
                                    in_=ps[:])
                        stats_and_gather(co)

                wpool.release()

                # ---- input activations are dead; swap left-stack pools
                if act_pool_in is not None:
                    act_pool_in.release()
                act_pool_out = tc.alloc_tile_pool(name=f"act{l}", bufs=1,
                                                  side="left")

                # ---- pools on DVE: depend only on raw, so they run during
                # the last chunk's collective. pooled/pm tiles reuse the
                # region wpool just freed.
                pooled = {}
                pm_pool = None
                if pool:
                    pm_pool = tc.alloc_tile_pool(name=f"pm{l}", bufs=1,
                                                 side="right")
                    for co in range(co_ch):
                        pooled[co] = pm_pool.tile(
                            [128, (hout // 2) * (wout // 2) * B], F32,
                            tag=f"p{co}", name=f"pool{l}_{co}")
                    pm = pm_pool.tile([128, (hout // 2) * wout * B],
                                      F32, tag="pm", name=f"pm{l}")
                    for co in range(co_ch):
                        rv = raw[co][:].rearrange(
                            "p (h two w b) -> p h two w b",
                            h=hout // 2, two=2, w=wout, b=B)
                        nc.vector.tensor_max(out=pm[:], in0=rv[:, :, 0, :, :],
                                             in1=rv[:, :, 1, :, :])
                        pv = pm[:].rearrange(
                            "p (h w two b) -> p h w two b",
                            h=hout // 2, w=wout // 2, two=2, b=B)
                        nc.vector.tensor_max(out=pooled[co][:],
                                             in0=pv[:, :, :, 0, :],
                                             in1=pv[:, :, :, 1, :])

                # ---- per chunk: gathered stats -> local combine -> a, c ->
                # transforms. Interleaved per chunk so chunk co's transforms
                # are never queued behind a later chunk's collective wait:
                # chunks 0..co_ch-2 drain during the last chunk's collective.
                a_all = const_pool.tile([128, co_ch], F32, tag=f"a{l}",
                                        name=f"a{l}")
                c_all = const_pool.tile([128, co_ch], F32, tag=f"c{l}",
                                        name=f"c{l}")
                g_t = const_pool.tile([128, 16 * co_ch], F32, tag=f"gt{l}",
                                      name=f"g{l}")
                u_t = const_pool.tile([128, 8 * co_ch], F32, tag=f"ut{l}",
                                      name=f"u{l}")
                ho = hout // 2 if pool else hout
                wo = wout // 2 if pool else wout
                src = pooled if pool else raw
                act_out = {}
                for co in range(co_ch):
                    act_out[co] = act_pool_out.tile([128, ho * wo * B], F16,
                                                    tag=f"h{co}",
                                                    name=f"a{l}_{co}")
                # row blocks: (start, nrows)
                blocks = []
                r = 0
                while r < ho:
                    nr = min(4, ho - r)
                    blocks.append((r, nr))
                    r += nr
                for co in range(co_ch):
                    g_c = g_t[:, co * 16:(co + 1) * 16]
                    u_c = u_t[:, co * 8:(co + 1) * 8]
                    if collectives:
                        nc.sync.dma_start(
                            out=g_c,
                            in_=AP(cc_out[(l, co)], 0,
                                   [[2, 128], [256, 8], [1, 2]]))
                    else:
                        nc.vector.memset(g_c, 0.0)
                        nc.sync.dma_start(
                            out=g_c[:, 0:2], in_=cc_in[(l, co)][:, :])
                    # gathered pairs are (mean_c, var_c): first make the
                    # var slots var_c + mean_c^2 ...
                    gv = g_c.rearrange("p (k two) -> p k two", two=2)
                    nc.vector.tensor_mul(out=u_c, in0=gv[:, :, 0],
                                         in1=gv[:, :, 0])
                    nc.vector.tensor_add(out=gv[:, :, 1], in0=gv[:, :, 1],
                                         in1=u_c)
                    # ... then sum the 8 (mean, E2) pairs: 16 -> 8 -> 4 -> 2
                    nc.vector.tensor_add(out=u_c, in0=g_c[:, 0:8],
                                         in1=g_c[:, 8:16])
                    nc.vector.tensor_add(out=u_c[:, 0:4], in0=u_c[:, 0:4],
                                         in1=u_c[:, 4:8])
                    nc.vector.tensor_add(out=u_c[:, 0:2], in0=u_c[:, 0:2],
                                         in1=u_c[:, 2:4])
                    s1 = u_c[:, 0:1]
                    s2 = u_c[:, 1:2]
                    # nm = -mean; nvar = mean^2 - E2 = -var
                    nm = u_c[:, 2:3]
                    nvar = u_c[:, 3:4]
                    nc.vector.tensor_scalar_mul(nm, s1, -1.0 / N_CORES)
                    nc.vector.tensor_scalar_mul(nvar, s2, 1.0 / N_CORES)
                    # nvar := (nm*nm) - E2/8 = mean^2 - E[x^2] = -var
                    nc.vector.scalar_tensor_tensor(
                        out=nvar, in0=nm, scalar=nm, in1=nvar,
                        op0=ALU.mult, op1=ALU.subtract)
                    sd = u_c[:, 4:5]
                    nc.scalar.activation(out=sd, in_=nvar, func=AF.Sqrt,
                                         bias=eps_t[:], scale=-1.0)
                    rc = u_c[:, 5:6]
                    nc.vector.reciprocal(out=rc, in_=sd)
                    a_c = a_all[:, co:co + 1]
                    c_c = c_all[:, co:co + 1]
                    nc.vector.tensor_mul(out=a_c, in0=rc,
                                         in1=gb_all[:, co:co + 1])
                    nc.vector.tensor_mul(out=c_c, in0=nm, in1=a_c)
                    nc.vector.tensor_add(out=c_c, in0=c_c,
                                         in1=gb_all[:, co_ch + co:
                                                    co_ch + co + 1])
                    for bi, (r0, nr) in enumerate(blocks):
                        s = src[co][:, r0 * wo * B:(r0 + nr) * wo * B]
                        d = act_out[co][:, r0 * wo * B:(r0 + nr) * wo * B]
                        if (bi + co) % 2 == 0:
                            nc.scalar.activation(out=d, in_=s, func=AF.Relu,
                                                 bias=c_c, scale=a_c)
                        else:
                            nc.vector.tensor_scalar(
                                out=d, in0=s, scalar1=a_c, scalar2=c_c,
                                op0=ALU.mult, op1=ALU.add)
                            nc.vector.tensor_scalar_max(d, d, 0.0)

                if pm_pool is not None:
                    pm_pool.release()
                raw_pool.release()
                stats_pool.release()
                return act_out, act_pool_out

            act = {0: ic}
            act, act_pool = conv_layer(1, act, im2col_pool)

            fc_a = None
            fw1_sb = []
            fw2_sb = []
            fw3_sb = []
            ident_sb = None
            for l in range(2, min(upto, 6) + 1):
                hook = None
                if l == 5 and upto >= 9:
                    # preload ALL fc weights behind L5/L6 compute; their
                    # DMAs are emitted after L5's own weight DMAs (hook) so
                    # they never delay the conv stream, and the dedicated
                    # pool keeps them clear of the raw6/pool WAR region
                    fc_a = tc.alloc_tile_pool(name="fc_a", bufs=1,
                                              side="right")
                    for kc in range(36):
                        fw1_sb.append(fc_a.tile([128, 1024], F16,
                                                tag=f"fw1_{kc}",
                                                name=f"fw1_{kc}"))
                    for kc in range(8):
                        fw2_sb.append(fc_a.tile([128, 1024], F16,
                                                tag=f"fw2_{kc}",
                                                name=f"fw2_{kc}"))
                    for kc in range(8):
                        fw3_sb.append(fc_a.tile([128, 10], F16,
                                                tag=f"fw3_{kc}",
                                                name=f"fw3_{kc}"))
                    ident_sb = fc_a.tile([32, 32], F32, tag="ident",
                                         name="ident")

                    def hook():
                        for kc in range(36):
                            nc.sync.dma_start(
                                out=fw1_sb[kc][:],
                                in_=fw1p[kc * 128:(kc + 1) * 128, :])
                        for kc in range(8):
                            nc.sync.dma_start(
                                out=fw2_sb[kc][:],
                                in_=fw2p[kc * 128:(kc + 1) * 128, :])
                        for kc in range(8):
                            nc.sync.dma_start(
                                out=fw3_sb[kc][:],
                                in_=fw3p[kc * 128:(kc + 1) * 128, :])
                        nc.sync.dma_start(out=ident_sb[:], in_=identp[:, :])
                act, act_pool = conv_layer(l, act, act_pool, post_w_hook=hook)

            if upto < 9:
                (_, _, cout, _, hout, pool) = CONV[upto - 1]
                ho = hout // 2 if pool else hout
                co_ch = cout // 128
                if not dbg:
                    nc.sync.dma_start(out=dbg_t[:, :], in_=act[0][:, :B])
                else:
                    for co in range(co_ch):
                        nc.sync.dma_start(
                            out=dbg_t[co * 128:(co + 1) * 128, :],
                            in_=act[co][:])
            else:
                # ---------------- FC stack ----------------
                fc_pool = tc.alloc_tile_pool(name="fc", bufs=1, side="right")

                # fc1 (transposed form): out[b, o] with the activation chunk
                # as the stationary operand and fw1 rows streaming (N=512) --
                # 72 big matmuls instead of 288 N=32 ones.
                # lhsT kc = (co*3 + r)*3 + w: act6 co tiles [128, 9*B]
                lhs1 = []
                for co in range(4):
                    for rw in range(9):
                        lhs1.append(act[co][:, rw * B:(rw + 1) * B])

                def fc_T(lhs_list, w_sb, n_out, tagp):
                    """Transposed-form FC + relu + PE transpose back.
                    lhs_list: kc -> [128, 32] fp16 chunks of the input (k on
                    partitions). w_sb: kc -> [128, n_out] weight rows (rhs).
                    Returns list over 8 o-chunks of [128, 32] fp16 tiles
                    (o on partitions)."""
                    nkc = len(lhs_list)
                    ysb = fc_pool.tile([32, n_out], F32, tag=f"{tagp}y",
                                       name=f"{tagp}y")
                    # oh groups interleaved across kc so the late (last-
                    # chunk-dependent) kcs are reached as late as possible
                    pss = [psum_pool.tile([32, 512], F32, tag="ps",
                                          name=f"{tagp}ps{oh}")
                           for oh in range(n_out // 512)]
                    for kc, lh in enumerate(lhs_list):
                        for oh, ps in enumerate(pss):
                            nc.tensor.matmul(
                                out=ps[:], lhsT=lh,
                                rhs=w_sb[kc][:, oh * 512:(oh + 1) * 512],
                                start=(kc == 0), stop=(kc == nkc - 1))
                    for oh, ps in enumerate(pss):
                        nc.scalar.activation(
                            out=ysb[:, oh * 512:(oh + 1) * 512], in_=ps[:],
                            func=AF.Relu)
                    outs = []
                    for kc2 in range(n_out // 128):
                        trp = psum_pool.tile([128, 32], F32, tag="ps",
                                             name=f"{tagp}tr{kc2}")
                        nc.tensor.transpose(
                            out=trp[:],
                            in_=ysb[:, kc2 * 128:(kc2 + 1) * 128],
                            identity=ident_sb[:])
                        t = fc_pool.tile([128, 32], F16, tag=f"{tagp}T{kc2}",
                                         name=f"{tagp}T{kc2}")
                        nc.vector.tensor_copy(out=t[:], in_=trp[:])
                        outs.append(t)
                    return outs

                y1T = fc_T(lhs1, fw1_sb, 1024, "y1")
                y2T = fc_T([t[:] for t in y1T], fw2_sb, 1024, "y2")

                ps = psum_pool.tile([10, B], F32, tag="ps", name="ps10")
                for kc, rh in enumerate([t[:] for t in y2T]):
                    lhsT = fw3_sb[kc][:, :]
                    nc.tensor.matmul(out=ps[:], lhsT=lhsT, rhs=rh,
                                     start=(kc == 0), stop=(kc == 7))
                sig = fc_pool.tile([10, B], F32, tag="sig", name="sig")
                nc.scalar.activation(out=sig[:], in_=ps[:], func=AF.Sigmoid)
                nc.sync.dma_start(out=out_ext[:, :], in_=sig[:])
                fc_pool.release()
                if fc_a is not None:
                    fc_a.release()
            act_pool.release()

        for _rep in range(reps):
            body()

        psum_pool.release()
        stage_pool.release()
        const_pool.release()

    nc.compile()
    return nc


# ---------------- host-side input prep ----------------

def _f16(x):
    return np.asarray(x, np.float32).astype(np.float16)


def prep_inputs(inputs):
    """Full reference inputs -> per-core input maps."""
    x = np.asarray(inputs["x"], np.float32).reshape(256, 28 * 28)
    shared = {}
    w1 = np.sign(np.asarray(inputs["w1"], np.float32)).reshape(128, 9)
    w1rep = np.zeros((128, 128), np.float32)
    for gi in range(4):
        w1rep[32 * gi:32 * gi + 9, :] = w1.T
    shared["w1p"] = _f16(w1rep)
    for l in (2, 3, 4, 5, 6):
        w = np.sign(np.asarray(inputs[f"w{l}"], np.float32))
        cout, cin = w.shape[0], w.shape[1]
        ci_ch, co_ch = cin // 128, cout // 128
        s = w.reshape(co_ch, 128, ci_ch, 128, 9)
        s = np.transpose(s, (2, 3, 0, 4, 1))  # [ci_ch,128,co_ch,9,128]
        shared[f"w{l}p"] = _f16(s.reshape(ci_ch * 128, co_ch * 9 * 128).copy())
    for l in range(1, 7):
        shared[f"g{l}p"] = np.asarray(inputs[f"g{l}"], np.float32).reshape(-1, 1)
        shared[f"b{l}p"] = np.asarray(inputs[f"b{l}"], np.float32).reshape(-1, 1)
    fw1 = np.sign(np.asarray(inputs["fw1"], np.float32))  # [1024, 4608]
    v = fw1.reshape(1024, 4, 128, 9)
    v = np.transpose(v, (1, 3, 2, 0))  # [4, 9, 128, 1024], kc = cc*9+hw
    shared["fw1p"] = _f16(v.reshape(36 * 128, 1024).copy())
    fw2 = np.sign(np.asarray(inputs["fw2"], np.float32))
    shared["fw2p"] = _f16(fw2.T.reshape(8 * 128, 1024).copy())
    fw3 = np.sign(np.asarray(inputs["fw3"], np.float32))
    shared["fw3p"] = _f16(fw3.T.reshape(8 * 128, 10).copy())
    shared["identp"] = np.eye(32, dtype=np.float32)

    per_core = []
    for c in range(N_CORES):
        xs = x[c * B:(c + 1) * B].T.copy()  # [784, B]
        m = dict(shared)
        m["xp"] = _f16(xs)
        per_core.append(m)
    return per_core


_NC_CACHE = {}


def kernel(**inputs) -> np.ndarray:
    if "nc" not in _NC_CACHE:
        _NC_CACHE["nc"] = build(upto=9)
    nc = _NC_CACHE["nc"]
    per_core = prep_inputs(inputs)
    last_err = None
    for _attempt in range(3):
        try:
            res = run_bass_kernel_spmd(nc, per_core, list(range(N_CORES)))
            break
        except Exception as e:  # transient device wedge: wait and retry
            last_err = e
            import time as _time
            _time.sleep(15)
    else:
        raise last_err
    out = np.empty((256, 10), np.float32)
    for c in range(N_CORES):
        out[c * B:(c + 1) * B] = res.results[c]["out"].T
    return out


if __name__ == "__main__":
    import reference as R
    inputs = R.setup_inputs()
    got = kernel(**{k: np.asarray(v) for k, v in inputs.items()})
    exp = np.asarray(R.reference(**inputs))
    err = np.abs(got - exp)
    rel = np.linalg.norm(got - exp) / np.linalg.norm(exp)
    print(f"absmax {err.max():.3e}  rel {rel:.3e}")
